# revision 39
# baseline (speedup 1.0000x reference)
"""Trainium2 Bass kernel for nn_BiGNN_53772990546511.

Restructured v2 (validated in numpy, global l2 rel err ~3.4e-4):
  - relu(elu(x)) == relu(x) exactly; loc rows collapse to broadcast rows from
    day 1 on; days 2-4 are rank-2 row algebra.
  - Day-1 attention: exp(leaky(f1+f2)) = max(exp(f1)exp(f2), exp(.2f1)exp(.2f2))
    is rank-1 per branch -> built with vector ops from per-loc / per-user
    exp factors (bias -5 each side).  No big scalar-engine Exp tiles.
  - Phase2/6 run transposed ([feat, user]) so the contraction is over locs
    with N=512 moving width; user-space results recovered with PE transposes.
  - A is shipped as fp8 e4m3 raw counts (exact) with 1/cnt applied on device;
    M as f16 {0,1,2}.  Outputs are written f16 (harness casts to f32).
  - Z-normalization scaled by CZ=0.25 to keep 1/Z inside f16 range.

Sharding: 8 cores = 4 batch pairs, SPMD-uniform; odd cores get user-axis
rotated (by 512) host tensors so local users 0..511 are global 512..1023.
"""
import numpy as np

N_USER = 1024
N_LOC = 1024
DM = 256
HD = 256
B = 4
D = 5
E = 4096
ALPHA = 0.2
CZ = 0.25          # Z scale: wh stored *1/CZ, recipZ stored CZ/Z
FB = -5.0          # per-side exp bias
P = 128
NCORES = 8

_CACHE = {}


# --------------------------------------------------------------------------
# Workarounds for this walrus build's 1-sync-wait-per-instruction limit.
# --------------------------------------------------------------------------
def _apply_tile_patch():
    import concourse.tile as tile
    from concourse.tile_sem_assignment import tick_to_sem

    if not getattr(tile.TileContext, "_drain_patched", False):
        def _patched(self, tick_clock, wait_clock):
            nc = self.nc
            gc = tick_clock.global_clock
            for proc, sem in self.sems.allocated().items():
                t = gc[proc]
                if t and t > 0:
                    nc.sync.nop().wait_op(sem, tick_to_sem(t, proc), "sem-ge")
            nc.sync.drain()
            nc.all_engine_barrier()
            popped = nc._tile_sem_poison_stack.pop()
            assert popped is self._sem_poison
            nc.clear_and_free_semaphores(list(self.sems.allocated().values()))
            nc.all_engine_barrier()

        tile.TileContext._drain_and_barrier = _patched
        tile.TileContext._drain_patched = True

    import json as _json
    import concourse.bass_utils as _bu
    import concourse.bass2jax as _b2j

    if not getattr(_bu, "_wait_split_patched", False):
        _orig_compile = _bu.compile_bir_kernel

        def _split_waits(bir_json):
            j = _json.loads(bir_json)
            nid = [0]
            for fn in j.get("functions", []):
                for bb in fn.get("blocks", []):
                    out = []
                    for inst in bb.get("instructions", []):
                        si = inst.get("sync_info") or {}
                        ow = si.get("on_wait") or []
                        if len(ow) > 1:
                            for w in ow[:-1]:
                                nid[0] += 1
                                out.append({
                                    "debug": inst.get("debug", 0),
                                    "engine": inst.get("engine", "SP"),
                                    "ins": [],
                                    "name": f"WSPL-{nid[0]}",
                                    "opcode": "NoOp",
                                    "outs": [],
                                    "sync_info": {"on_update": [],
                                                  "on_wait": [w]},
                                })
                            si["on_wait"] = [ow[-1]]
                        out.append(inst)
                    bb["instructions"] = out
            return _json.dumps(j).encode()

        def _patched_compile(bir_json, tmpdir, neff_name="file.neff"):
            return _orig_compile(_split_waits(bir_json), tmpdir,
                                 neff_name=neff_name)

        _bu.compile_bir_kernel = _patched_compile
        _b2j.compile_bir_kernel = _patched_compile
        _bu._wait_split_patched = True


def _build_nc():
    import contextlib
    import concourse.bass as bass
    import concourse.tile as tile
    from concourse import mybir
    from concourse.masks import make_identity

    _apply_tile_patch()
    f32 = mybir.dt.float32
    f16 = mybir.dt.float16
    f8 = mybir.dt.float8e4
    AF = mybir.ActivationFunctionType
    OP = mybir.AluOpType

    nc = bass.Bass()

    # ---------------- DRAM tensors ----------------
    d_A8T = nc.dram_tensor("A8T", [N_LOC, N_USER], f8, kind="ExternalInput")
    d_MT = nc.dram_tensor("MT", [N_LOC, N_USER], f16, kind="ExternalInput")
    d_xloc16 = nc.dram_tensor("xloc16", [N_LOC, DM], f16, kind="ExternalInput")
    d_xlocT16 = nc.dram_tensor("xlocT16", [DM, N_LOC], f16,
                               kind="ExternalInput")
    d_W16 = nc.dram_tensor("W16", [DM, HD], f16, kind="ExternalInput")
    d_WT16 = nc.dram_tensor("WT16", [HD, DM], f16, kind="ExternalInput")
    d_acolP = nc.dram_tensor("acolP", [P, 4], f16, kind="ExternalInput")
    d_recip = nc.dram_tensor("recip", [1, N_USER], f16, kind="ExternalInput")
    d_gtri = nc.dram_tensor("gtri", [N_LOC, 3], f16, kind="ExternalInput")
    d_recipcol = nc.dram_tensor("recipcol", [P, 4], f32,
                                kind="ExternalInput")
    d_nothas = nc.dram_tensor("nothas", [1, N_USER], f16, kind="ExternalInput")
    d_nothascol = nc.dram_tensor("nothascol", [P, 4], f32,
                                 kind="ExternalInput")
    d_nhcol = nc.dram_tensor("nhcol", [P, 1], f32, kind="ExternalInput")
    d_ner = nc.dram_tensor("ner", [P, 6], f32, kind="ExternalInput")
    d_hn2 = nc.dram_tensor("hn2", [2, 3 * 512], f16, kind="ExternalInput")
    d_out = nc.dram_tensor("out", [D, 1536, HD], f16, kind="ExternalOutput")

    with tile.TileContext(nc) as tc:
        with contextlib.ExitStack() as ctx:
            persist = ctx.enter_context(tc.tile_pool(name="persist", bufs=1))
            work = ctx.enter_context(tc.tile_pool(name="work", bufs=1))
            psAcc = ctx.enter_context(
                tc.tile_pool(name="psAcc", bufs=1, space="PSUM"))
            psZb = ctx.enter_context(
                tc.tile_pool(name="psZb", bufs=1, space="PSUM"))
            psF = ctx.enter_context(
                tc.tile_pool(name="psF", bufs=1, space="PSUM"))
            psT16 = ctx.enter_context(
                tc.tile_pool(name="psT16", bufs=2, space="PSUM"))

            def big_load(eng, dst, dram, t):
                src = dram.rearrange("(t p) u -> p t u", p=P)
                eng.dma_start(out=dst[:].rearrange("p (t u) -> p t u", t=t),
                              in_=src)

            # ------------- input loads (two DMA queues) -------------
            # sync queue: phase-1 criticals, then A8, then small rows
            acolP = persist.tile([P, 4], f16, name="acolP")
            nc.sync.dma_start(out=acolP[:], in_=d_acolP[:])
            WT16 = persist.tile([P, 2 * DM], f16, name="WT16")
            big_load(nc.sync, WT16, d_WT16[:], 2)
            Wext = persist.tile([P, 2 * 258], f16, name="Wext")
            for kt in range(2):
                nc.sync.dma_start(
                    out=Wext[:, kt * 258:kt * 258 + 256],
                    in_=d_W16[kt * P:(kt + 1) * P, :])
            xlocT16 = persist.tile([P, 2 * N_LOC], f16, name="xlocT16")
            big_load(nc.sync, xlocT16, d_xlocT16[:], 2)
            A8 = persist.tile([P, 8 * N_USER], f8, name="A8")
            gtri = persist.tile([P, 8 * 3], f16, name="gtri")
            recipcol = persist.tile([P, 4], f32, name="recipcol")
            nothasrow = persist.tile([1, N_USER], f16, name="nothasrow")
            nothascol = persist.tile([P, 4], f32, name="nothascol")
            nhcol = persist.tile([P, 1], f32, name="nhcol")
            nercols = persist.tile([P, 6], f32, name="nercols")
            hn2 = persist.tile([2, 3 * 512], f16, name="hn2")
            # scalar queue: xloc16 (early; day0 locs written from it), MT
            xloc16 = persist.tile([P, 8 * DM], f16, name="xloc16")
            big_load(nc.scalar, xloc16, d_xloc16[:], 8)
            reciprow = persist.tile([1, N_USER], f16, name="reciprow")
            MT = persist.tile([P, 8 * N_USER], f16, name="MT")

            # ------------- constants -------------
            ident16 = persist.tile([P, P], f16, name="ident16")
            make_identity(nc, ident16[:])
            ones16 = persist.tile([P, 1], f16, name="ones16")
            nc.vector.memset(ones16[:], 1.0)
            one11 = persist.tile([1, 1], f16, name="one11")
            nc.vector.memset(one11[:], 1.0)
            onesrow16 = persist.tile([1, P], f16, name="onesrow16")
            nc.vector.memset(onesrow16[:], 1.0)
            fbcol = persist.tile([P, 1], f32, name="fbcol")
            nc.vector.memset(fbcol[:], FB)
            fb11 = persist.tile([1, 1], f32, name="fb11")
            nc.vector.memset(fb11[:], FB)

            def bcast_mm(dst, row_ap, n, copy_eng=None):
                # physical partition-broadcast of a [1,n] f16 row via K=1
                # matmuls (PE) + PSUM->SBUF copies
                for ci, c0 in enumerate(range(0, n, 512)):
                    w = min(512, n - c0)
                    ps = psF.tile([P, 512], f32, name="bc", tag="sf32")
                    nc.tensor.matmul(ps[:, 0:w], onesrow16[:],
                                     row_ap[0:1, c0:c0 + w],
                                     start=True, stop=True)
                    if copy_eng is nc.vector or ci % 2 == 1:
                        nc.vector.tensor_copy(dst[:, c0:c0 + w], ps[:, 0:w])
                    else:
                        nc.scalar.activation(dst[:, c0:c0 + w], ps[:, 0:w],
                                             AF.Copy)

            def Wk(kt, mh):
                return Wext[:, kt * 258 + mh * P:kt * 258 + (mh + 1) * P]

            def A8s(lt, sl):
                return A8[:, lt * N_USER:(lt + 1) * N_USER][:, sl]

            def Ms(lt):
                return MT[:, lt * N_USER:(lt + 1) * N_USER]


            # ---------------- phase W: wa pairs + Wext ----------------
            wapair = [persist.tile([P, 2], f16, name=f"wap{i}")
                      for i in range(2)]
            for mh in range(2):
                ps = psF.tile([P, 512], f32, name="pswa", tag="sf32")
                ps = ps[:, 2 * mh:2 * mh + 2]
                for kt in range(2):
                    nc.tensor.matmul(
                        ps[:],
                        WT16[:, kt * DM + mh * P:kt * DM + (mh + 1) * P],
                        acolP[:, 2 * kt:2 * kt + 2],
                        start=(kt == 0), stop=(kt == 1))
                nc.vector.tensor_copy(wapair[mh][:], ps[:])
            for kt in range(2):
                nc.vector.tensor_copy(Wext[:, kt * 258 + 256:kt * 258 + 258],
                                      wapair[kt][:])

            # ---------------- phase 1: wh + per-loc factors ----------------
            wh16 = persist.tile([P, 8 * HD], f16, name="wh16")
            xw1c = persist.tile([P, 8], f16, name="xw1c")
            xw2c = persist.tile([P, 8], f32, name="xw2c")
            u1col = persist.tile([P, 8], f32, name="u1col")
            u2col = persist.tile([P, 8], f32, name="u2col")
            for lt in range(8):
                ps = psAcc.tile([P, 512], f32, name="whx",
                                tag=f"acc{(lt % 4) // 2}{lt % 2}")
                ps = ps[:, 0:258]
                for kt in range(2):
                    nc.tensor.matmul(
                        ps[:],
                        xlocT16[:, kt * N_LOC + lt * P:
                                kt * N_LOC + (lt + 1) * P],
                        Wext[:, kt * 258:(kt + 1) * 258],
                        start=(kt == 0), stop=(kt == 1))
                nc.vector.tensor_scalar(
                    out=wh16[:, lt * HD:(lt + 1) * HD], in0=ps[:, 0:256],
                    scalar1=1.0 / CZ, scalar2=0.0, op0=OP.mult, op1=OP.add)
                nc.vector.tensor_copy(xw1c[:, lt:lt + 1], ps[:, 256:257])
                nc.vector.tensor_copy(xw2c[:, lt:lt + 1], ps[:, 257:258])
            nc.scalar.activation(u1col[:], xw2c[:], AF.Exp, bias=fbcol[:],
                                 scale=1.0)
            nc.scalar.activation(u2col[:], xw2c[:], AF.Exp, bias=fbcol[:],
                                 scale=ALPHA)

            # ---------------- phase 2: xuT (local half) + g-sums ----------------
            big_load(nc.sync, A8, d_A8T[:], 8)
            big_load(nc.scalar, gtri, d_gtri[:], 8)
            psxu = [psAcc.tile([P, 512], f32, name=f"xu{fh}",
                               tag=f"acc0{fh}") for fh in range(2)]
            psG = [psF.tile([P, 512], f32, name="psG0", tag="sf32"),
                   psZb.tile([P, 512], f32, name="psG1", tag="zz")]
            for lt in range(8):
                for fh in range(2):
                    nc.tensor.matmul(
                        psxu[fh][:],
                        xloc16[:, lt * DM + fh * P:lt * DM + (fh + 1) * P],
                        A8s(lt, slice(0, 512)),
                        start=(lt == 0), stop=(lt == 7))
                    nc.tensor.matmul(
                        psG[fh][:, 0:3],
                        xloc16[:, lt * DM + fh * P:lt * DM + (fh + 1) * P],
                        gtri[:, lt * 3:(lt + 1) * 3],
                        start=(lt == 0), stop=(lt == 7),
                        skip_group_check=True)
            nc.scalar.dma_start(out=recipcol[:], in_=d_recipcol[:])
            nc.scalar.dma_start(out=nothascol[:], in_=d_nothascol[:])
            nc.scalar.dma_start(out=nothasrow[:], in_=d_nothas[:])
            nc.scalar.dma_start(out=nhcol[:], in_=d_nhcol[:])
            nc.scalar.dma_start(out=reciprow[:], in_=d_recip[:])
            big_load(nc.scalar, MT, d_MT[:], 8)
            # raw f16 copies (1/cnt applied after transpose, per-user)
            xu16T = persist.tile([P, 2 * 512], f16, name="xu16T")
            for fh in range(2):
                nc.vector.tensor_copy(xu16T[:, fh * 512:(fh + 1) * 512],
                                      psxu[fh][:])
            # mwe/sxu/sxl columns from the g matmul
            mwe16 = [persist.tile([P, 1], f16, name=f"mwe16{fh}")
                     for fh in range(2)]
            mcol16 = [persist.tile([P, 1], f16, name=f"mcol16{fh}")
                      for fh in range(2)]
            for fh in range(2):
                nc.vector.tensor_copy(mwe16[fh][:], psG[fh][:, 0:1])
                t1 = work.tile([P, 1], f32, name=f"t1{fh}", tag=f"t1{fh}")
                nc.vector.tensor_scalar(
                    out=t1[:], in0=psG[fh][:, 2:3], scalar1=2.0 / 3072.0,
                    scalar2=0.0, op0=OP.mult, op1=OP.add)
                nc.vector.scalar_tensor_tensor(
                    out=t1[:], in0=psG[fh][:, 1:2], scalar=1.0 / 3072.0,
                    in1=t1[:], op0=OP.mult, op1=OP.add)
                nc.vector.scalar_tensor_tensor(
                    out=mcol16[fh][:], in0=psG[fh][:, 0:1], scalar=nhcol[:],
                    in1=t1[:], op0=OP.mult, op1=OP.add)

            # mwe row + broadcast
            mwerow = persist.tile([1, HD], f16, name="mwerow")
            for fh in range(2):
                pst = psT16.tile([P, 256], f16, name="pst1", tag="t16")
                pst = pst[0:1, 0:P]
                nc.tensor.transpose(pst, mwe16[fh][:], ident16[:])
                nc.vector.tensor_copy(mwerow[0:1, fh * P:(fh + 1) * P], pst)
            mweB = persist.tile([P, HD], f16, name="mweB")
            bcast_mm(mweB, mwerow[:], HD)

            # day-0 user rows: transpose local half + fixup
            day0u = work.tile([P, 4 * HD], f16, name="day0u", tag="day0u")
            for fh in range(2):
                for ut in range(4):
                    pst = psT16.tile([P, 256], f16, name="pstT", tag="t16")
                    pst = pst[:, 0:P]
                    nc.tensor.transpose(
                        pst,
                        xu16T[:, fh * 512 + ut * P:fh * 512 + (ut + 1) * P],
                        ident16[:])
                    tr = work.tile([P, P], f16, name="tr", tag="tr",
                                   bufs=2)
                    nc.scalar.activation(tr[:], pst, AF.Copy,
                                         scale=recipcol[:, ut:ut + 1])
                    nc.vector.scalar_tensor_tensor(
                        out=day0u[:, ut * HD + fh * P:ut * HD + (fh + 1) * P],
                        in0=mweB[:, fh * P:(fh + 1) * P],
                        scalar=nothascol[:, ut:ut + 1], in1=tr[:],
                        op0=OP.mult, op1=OP.add)
            nc.sync.dma_start(
                out=d_out[0, 0:512, :].rearrange("(t p) h -> p t h", p=P),
                in_=day0u[:].rearrange("p (t h) -> p t h", t=4))
            nc.scalar.dma_start(
                out=d_out[0, 512:1536, :].rearrange("(t p) h -> p t h", p=P),
                in_=xloc16[:].rearrange("p (t h) -> p t h", t=8))

            # ---------------- phase 3: mw0 / v1 / s1 ----------------
            mw0c16 = [persist.tile([P, 1], f16, name=f"mw0c{mh}")
                      for mh in range(2)]
            v1col16 = [persist.tile([P, 1], f16, name=f"v1c{mh}")
                       for mh in range(2)]
            for mh in range(2):
                ps = psF.tile([P, 512], f32, name="psmw", tag="sf32")
                ps = ps[:, mh:mh + 1]
                for kt in range(2):
                    nc.tensor.matmul(ps, Wk(kt, mh), mcol16[kt][:],
                                     start=(kt == 0), stop=(kt == 1))
                nc.vector.tensor_scalar(out=mw0c16[mh][:], in0=ps,
                                        scalar1=1.0 / CZ, scalar2=0.0,
                                        op0=OP.mult, op1=OP.add)
                nc.vector.tensor_scalar(out=v1col16[mh][:], in0=ps,
                                        scalar1=1.0, scalar2=0.0,
                                        op0=OP.mult, op1=OP.max)
            mw0row = persist.tile([1, HD], f16, name="mw0row")
            for mh in range(2):
                pst = psT16.tile([P, 256], f16, name="pst1", tag="t16")
                pst = pst[0:1, 0:P]
                nc.tensor.transpose(pst, mw0c16[mh][:], ident16[:])
                nc.vector.tensor_copy(mw0row[0:1, mh * P:(mh + 1) * P],
                                      pst)
            v1row = persist.tile([1, HD], f16, name="v1row")
            nc.vector.tensor_scalar(out=v1row[:], in0=mw0row[:], scalar1=CZ,
                                    scalar2=0.0, op0=OP.mult, op1=OP.max)
            s1_16 = persist.tile([1, 1], f32, name="s1_16")
            ps = psF.tile([P, 512], f32, name="pss1", tag="sf32")
            ps = ps[0:1, 0:2]
            for fh in range(2):
                nc.tensor.matmul(ps, mwe16[fh][:], wapair[fh][:],
                                 start=(fh == 0), stop=(fh == 1))
            nc.vector.tensor_copy(s1_16[:], ps[0:1, 0:1])

            # ---------------- phase 4: f1 + user factors ----------------
            v1exp = persist.tile([1, N_USER], f16, name="v1exp")
            v2exp = persist.tile([1, N_USER], f16, name="v2exp")
            for uc in range(2):
                usl = slice(uc * 512, (uc + 1) * 512)
                psf = psAcc.tile([P, 512], f32, name=f"psf{uc}",
                                 tag=f"acc1{uc}")
                psf = psf[0:1, 0:512]
                for lt in range(8):
                    nc.tensor.matmul(psf, xw1c[:, lt:lt + 1],
                                     A8s(lt, usl), start=(lt == 0),
                                     stop=(lt == 7))
                f1s = work.tile([1, 512], f32, name=f"f1s{uc}",
                                tag=f"f1s{uc}")
                nc.vector.tensor_tensor(out=f1s[:], in0=psf,
                                        in1=reciprow[0:1, usl], op=OP.mult)
                nc.vector.scalar_tensor_tensor(
                    out=f1s[:], in0=nothasrow[0:1, usl], scalar=s1_16[:],
                    in1=f1s[:], op0=OP.mult, op1=OP.add)
                nc.scalar.activation(v1exp[0:1, usl], f1s[:], AF.Exp,
                                     bias=fb11[:], scale=1.0)
                nc.scalar.activation(v2exp[0:1, usl], f1s[:], AF.Exp,
                                     bias=fb11[:], scale=ALPHA)
            V1B = persist.tile([P, N_USER], f16, name="V1B")
            bcast_mm(V1B, v1exp[:], N_USER)
            V2B = persist.tile([P, N_USER], f16, name="V2B")
            bcast_mm(V2B, v2exp[:], N_USER)

            # ---------------- phase 5+6: attention ----------------
            psN = [[psAcc.tile([P, 512], f32, name=f"num{fh}{uc}",
                                tag=f"acc{fh}{uc}") for uc in range(2)]
                   for fh in range(2)]
            zz = psZb.tile([P, 512], f32, name="zz", tag="zz")
            psz = [zz[32 * uc:32 * uc + 1, 0:512] for uc in range(2)]
            for lt in range(8):
                x1m = work.tile([P, N_USER], f16, name="x1m", tag="x1m",
                                bufs=2)
                nc.scalar.activation(x1m[:], V1B[:], AF.Copy,
                                     scale=u1col[:, lt:lt + 1])
                x2m = work.tile([P, N_USER], f16, name="x2m", tag="x2m",
                                bufs=2)
                nc.scalar.activation(x2m[:], V2B[:], AF.Copy,
                                     scale=u2col[:, lt:lt + 1])
                x2x = work.tile([P, N_USER], f16, name="x2x", tag="x2x",
                                bufs=2)
                nc.vector.tensor_tensor(out=x2x[:], in0=x2m[:], in1=x1m[:],
                                        op=OP.max)
                ptm = work.tile([P, N_USER], f16, name="ptm", tag="ptm",
                                bufs=3)
                nc.vector.tensor_tensor(out=ptm[:], in0=x2x[:], in1=Ms(lt),
                                        op=OP.mult)
                for fh in range(2):
                    for uc in range(2):
                        nc.tensor.matmul(
                            psN[fh][uc][:],
                            wh16[:, lt * HD + fh * P:lt * HD + (fh + 1) * P],
                            ptm[:, uc * 512:(uc + 1) * 512],
                            start=(lt == 0), stop=False)
                for uc in range(2):
                    nc.tensor.matmul(psz[uc], ones16[:],
                                     ptm[:, uc * 512:(uc + 1) * 512],
                                     start=(lt == 0), stop=False)
            # fixups: no-edge users attend uniformly -> mean Wh (mw0)
            for fh in range(2):
                for uc in range(2):
                    nc.tensor.matmul(
                        psN[fh][uc][:], mw0row[0:1, fh * P:(fh + 1) * P],
                        nothasrow[0:1, uc * 512:(uc + 1) * 512],
                        start=False, stop=True)
            for uc in range(2):
                nc.tensor.matmul(psz[uc], one11[:],
                                 nothasrow[0:1, uc * 512:(uc + 1) * 512],
                                 start=False, stop=True)

            # Z rows -> cols -> 1/Z -> back to row -> broadcast
            zrow16 = persist.tile([1, N_USER], f16, name="zrow16")
            for uc in range(2):
                nc.scalar.activation(
                    zrow16[0:1, uc * 512:(uc + 1) * 512], psz[uc], AF.Copy)
            zcol = persist.tile([P, 8], f32, name="zcol")
            for ut in range(8):
                pst = psT16.tile([P, 256], f16, name="pstc", tag="t16")
                pst = pst[:, 0:1]
                nc.tensor.transpose(pst,
                                    zrow16[0:1, ut * P:(ut + 1) * P],
                                    one11[:])
                nc.vector.tensor_copy(zcol[:, ut:ut + 1], pst)
            rzall = persist.tile([P, 8], f32, name="rzall")
            nc.vector.reciprocal(rzall[:], zcol[:])
            rz16 = persist.tile([P, 8], f16, name="rz16")
            nc.vector.tensor_scalar(out=rz16[:], in0=rzall[:], scalar1=CZ,
                                    scalar2=0.0, op0=OP.mult, op1=OP.add)
            rzrow16 = persist.tile([1, N_USER], f16, name="rzrow16")
            for ut in range(8):
                pst = psT16.tile([P, 256], f16, name="pstr", tag="t16")
                pst = pst[0:1, 0:P]
                nc.tensor.transpose(pst, rz16[:, ut:ut + 1], ident16[:])
                nc.vector.tensor_copy(rzrow16[0:1, ut * P:(ut + 1) * P], pst)
            rzB = persist.tile([P, N_USER], f16, name="rzB")
            bcast_mm(rzB, rzrow16[:], N_USER)
            # fused: h1uT = relu(numT) * rz  with free-axis sum -> scol
            h1uT = persist.tile([P, 2 * N_USER], f16, name="h1uT")
            scolp = [[work.tile([P, 1], f32, name=f"scp{fh}{uc}",
                                tag=f"scp{fh}{uc}") for uc in range(2)]
                     for fh in range(2)]
            for fh in range(2):
                for uc in range(2):
                    usl = slice(uc * 512, (uc + 1) * 512)
                    nc.vector.scalar_tensor_tensor(
                        out=h1uT[:, fh * N_USER + uc * 512:
                                 fh * N_USER + (uc + 1) * 512],
                        in0=psN[fh][uc][:], scalar=0.0, in1=rzB[:, usl],
                        op0=OP.max, op1=OP.mult,
                        accum_out=scolp[fh][uc][:])
            scur = [persist.tile([P, 1], f32, name=f"scur{fh}")
                    for fh in range(2)]
            for fh in range(2):
                nc.vector.tensor_tensor(out=scur[fh][:], in0=scolp[fh][0][:],
                                        in1=scolp[fh][1][:], op=OP.add)
            # day-1 user rows: transpose local half of h1uT
            day1u = work.tile([P, 4 * HD], f16, name="day1u", tag="day1u")
            for ut in range(4):
                for fh in range(2):
                    pst = psT16.tile([P, 256], f16, name="pstT", tag="t16")
                    pst = pst[:, 0:P]
                    nc.tensor.transpose(
                        pst,
                        h1uT[:, fh * N_USER + ut * P:
                             fh * N_USER + (ut + 1) * P],
                        ident16[:])
                    nc.vector.tensor_copy(
                        day1u[:, ut * HD + fh * P:ut * HD + (fh + 1) * P],
                        pst)
            nc.scalar.dma_start(
                out=d_out[1, 0:512, :].rearrange("(t p) h -> p t h", p=P),
                in_=day1u[:].rearrange("p (t h) -> p t h", t=4))

            # loc rows day>=1: physical [128,256] broadcast (PE+copy), then
            # one DMA with 8x free-dim replication (parallel SBUF reads)
            def bcast_loc(day, vrow, eng):
                vB = work.tile([P, HD], f16, name=f"vB{day}", tag="vB",
                               bufs=2)
                bcast_mm(vB, vrow, HD, copy_eng=nc.vector)
                ap = vB[:]
                srcap = bass.AP(tensor=ap.tensor, offset=ap.offset,
                                ap=[list(ap.ap[0]), [0, 8], list(ap.ap[1])])
                dst = d_out[day, 512:1536, :].rearrange("(t p) h -> p t h",
                                                        p=P)
                eng.dma_start(out=dst, in_=srcap)

            bcast_loc(1, v1row, nc.sync)
            nc.scalar.dma_start(out=nercols[:], in_=d_ner[:])
            nc.scalar.dma_start(out=hn2[:], in_=d_hn2[:])

            # ---------------- phase 7: days 2..4 ----------------
            vcol = v1col16  # [128,1] f16 x2 (feature halves)
            for day in (2, 3, 4):
                dd = day - 2
                pair = [work.tile([P, 2], f16, name=f"pair{day}{fh}",
                                  tag=f"pair{fh}", bufs=2)
                        for fh in range(2)]
                for fh in range(2):
                    nc.vector.tensor_copy(pair[fh][:, 0:1], vcol[fh][:])
                    t2 = work.tile([P, 1], f32, name=f"t2{day}{fh}",
                                   tag=f"t2{fh}", bufs=2)
                    nc.vector.tensor_scalar(out=t2[:], in0=scur[fh][:],
                                            scalar1=1.0 / 3072.0, scalar2=0.0,
                                            op0=OP.mult, op1=OP.add)
                    nc.vector.scalar_tensor_tensor(
                        out=pair[fh][:, 1:2], in0=vcol[fh][:],
                        scalar=2048.0 / 3072.0, in1=t2[:],
                        op0=OP.mult, op1=OP.add)
                rv = [work.tile([P, 2], f16, name=f"rv{day}{mh}",
                                tag=f"rv{mh}", bufs=2) for mh in range(2)]
                pswp = psF.tile([P, 512], f32, name="pswp", tag="sf32")
                for mh in range(2):
                    ps = pswp[:, 2 * mh:2 * mh + 2]
                    for kt in range(2):
                        nc.tensor.matmul(ps, Wk(kt, mh), pair[kt][:],
                                         start=(kt == 0), stop=(kt == 1),
                                         skip_group_check=True)
                    nc.vector.tensor_scalar(out=rv[mh][:], in0=ps,
                                            scalar1=1.0, scalar2=0.0,
                                            op0=OP.mult, op1=OP.max)
                rows2 = work.tile([2, HD], f16, name=f"rows{day}",
                                  tag="rows2", bufs=2)
                vnrow = work.tile([1, HD], f16, name=f"vnrow{day}",
                                  tag="vnrow", bufs=2)
                for mh in range(2):
                    pst = psT16.tile([P, 256], f16, name="pst2", tag="t16")
                    nc.tensor.transpose(pst[0:2, 0:P], rv[mh][:], ident16[:])
                    nc.vector.tensor_copy(rows2[:, mh * P:(mh + 1) * P],
                                          pst[0:2, 0:P])
                    nc.tensor.transpose(pst[0:1, P:2 * P], rv[mh][:, 1:2],
                                        ident16[:])
                    nc.vector.tensor_copy(vnrow[0:1, mh * P:(mh + 1) * P],
                                          pst[0:1, P:2 * P])
                dayu = work.tile([P, 4 * HD], f16, name=f"dayu{day}",
                                 tag="dayu", bufs=2)
                for ut in range(4):
                    ps = psAcc.tile([P, 512], f32, name="psou",
                                    tag=f"acc0{ut % 2}")
                    ps = ps[:, 0:HD]
                    nc.tensor.matmul(
                        ps, hn2[:, dd * 512 + ut * P:dd * 512 + (ut + 1) * P],
                        rows2[:], start=True, stop=True)
                    nc.vector.tensor_copy(dayu[:, ut * HD:(ut + 1) * HD],
                                          ps)
                eng = nc.sync if day % 2 == 0 else nc.scalar
                eng.dma_start(
                    out=d_out[day, 0:512, :].rearrange("(t p) h -> p t h",
                                                       p=P),
                    in_=dayu[:].rearrange("p (t h) -> p t h", t=4))
                bcast_loc(day, vnrow, nc.scalar if day % 2 == 0 else nc.sync)
                if day < 4:
                    vcol = [rv[mh][:, 1:2] for mh in range(2)]
                    nscur = [work.tile([P, 1], f32, name=f"ns{day}{fh}",
                                       tag=f"ns{fh}", bufs=2)
                             for fh in range(2)]
                    for fh in range(2):
                        t3 = work.tile([P, 1], f32, name=f"t3{day}{fh}",
                                       tag=f"t3{fh}", bufs=2)
                        nc.vector.tensor_scalar(
                            out=t3[:], in0=rv[fh][:, 1:2],
                            scalar1=nercols[:, dd * 2 + 1:dd * 2 + 2],
                            scalar2=0.0, op0=OP.mult, op1=OP.add)
                        nc.vector.scalar_tensor_tensor(
                            out=nscur[fh][:], in0=rv[fh][:, 0:1],
                            scalar=nercols[:, dd * 2:dd * 2 + 1], in1=t3[:],
                            op0=OP.mult, op1=OP.add)
                    scur = nscur

    return nc


def _host_prep(x_loc, mob_links, text_links, W, a):
    """Index-only preprocessing -> per-core input maps."""
    import ml_dtypes
    f8 = ml_dtypes.float8_e4m3

    x_loc = np.ascontiguousarray(x_loc, np.float32)
    W = np.ascontiguousarray(W, np.float32)
    a = np.ascontiguousarray(a, np.float32).reshape(2 * HD)
    mob = np.asarray(mob_links)
    text = np.asarray(text_links)

    a1, a2 = a[:HD], a[HD:]
    acolP = np.stack([a1[:P], a2[:P], a1[P:], a2[P:]], axis=1)
    shared = {
        "xloc16": x_loc.astype(np.float16),
        "xlocT16": np.ascontiguousarray(x_loc.T).astype(np.float16),
        "W16": W.astype(np.float16),
        "WT16": np.ascontiguousarray(W.T).astype(np.float16),
        "acolP": np.ascontiguousarray(acolP).astype(np.float16),
    }

    in_maps = []
    for c in range(NCORES):
        b, r = c // 2, c % 2
        rot = r * 512
        u0 = np.concatenate([mob[b, 0, :, 0], text[b, 0, :, 0]]).astype(
            np.int64)
        l0 = np.concatenate([mob[b, 0, :, 1], text[b, 0, :, 1]]).astype(
            np.int64)
        cnt = np.bincount(u0, minlength=N_USER).astype(np.float32)
        A = np.zeros((N_USER, N_LOC), np.float32)
        np.add.at(A, (u0, l0), 1.0)
        Mb = np.zeros((N_USER, N_LOC), np.float32)
        Tb = np.zeros((N_USER, N_LOC), np.float32)
        Mb[mob[b, 0, :, 0], mob[b, 0, :, 1]] = 1.0
        Tb[text[b, 0, :, 0], text[b, 0, :, 1]] = 1.0
        M = Mb + Tb
        recip = 1.0 / np.maximum(cnt, 1.0)
        has0 = (cnt > 0).astype(np.float32)
        n_with = max(float(has0.sum()), 1.0)
        nh_cnt = float(N_USER) - float(has0.sum())

        def rollu(x, axis=0):
            return np.roll(x, -rot, axis=axis)

        hn2 = np.zeros((2, 3 * 512), np.float32)
        ner = np.zeros((P, 6), np.float32)
        for dd in range(3):
            us = np.concatenate([mob[b, dd + 1, :, 0], text[b, dd + 1, :, 0]])
            hE = np.zeros(N_USER, np.float32)
            hE[us] = 1.0
            hloc = rollu(hE)[:512]
            hn2[0, dd * 512:(dd + 1) * 512] = hloc
            hn2[1, dd * 512:(dd + 1) * 512] = 1.0 - hloc
            ner[:, dd * 2] = hE.sum()
            ner[:, dd * 2 + 1] = N_USER - hE.sum()

        nothas_r = rollu(1.0 - has0)
        m = dict(shared)
        m.update({
            "A8T": np.ascontiguousarray(rollu(A, 0).T).astype(f8),
            "MT": np.ascontiguousarray(rollu(M, 0).T).astype(np.float16),
            "recip": rollu(recip)[None, :].astype(np.float16),
            "gtri": np.stack([
                (has0 * recip / n_with) @ A,
                recip @ A,
                np.ones(N_LOC, np.float32)], axis=1).astype(np.float16),
            "recipcol": np.ascontiguousarray(
                rollu(recip)[:512].reshape(4, P).T).astype(np.float32),
            "nothas": nothas_r[None, :].astype(np.float16),
            "nothascol": np.ascontiguousarray(
                nothas_r[:512].reshape(4, P).T).astype(np.float32),
            "nhcol": np.full((P, 1), nh_cnt / 3072.0, np.float32),
            "ner": ner,
            "hn2": hn2.astype(np.float16),
        })
        in_maps.append(m)
    return in_maps


def kernel(**inputs):
    from concourse.bass_utils import run_bass_kernel_spmd

    if "nc" not in _CACHE:
        _CACHE["nc"] = _build_nc()
    nc = _CACHE["nc"]

    in_maps = _host_prep(inputs["x_loc"], inputs["mob_links"],
                         inputs["text_links"], inputs["W"], inputs["a"])
    res = run_bass_kernel_spmd(nc, in_maps, core_ids=list(range(NCORES)))

    out = np.zeros((B, D, N_USER + 2 * N_LOC, HD), np.float32)
    for c in range(NCORES):
        b, r = c // 2, c % 2
        o = np.asarray(res.results[c]["out"], np.float32)
        out[b, :, r * 512:(r + 1) * 512, :] = o[:, 0:512, :]
        out[b, :, N_USER + r * N_LOC:N_USER + (r + 1) * N_LOC, :] = \
            o[:, 512:1536, :]
    return out
